# revision 3
# baseline (speedup 1.0000x reference)
"""TRN2 Bass kernel for nn_GCNEModel (3-layer GCN + dense head), 8 NeuronCores.

Pair-sharded design: cores (2k, 2k+1) jointly process samples (2k, 2k+1).
Each core owns HALF the graph's nodes (round-robin by in-degree rank) and
aggregates all edges into its half, for BOTH samples of the pair, packed
into one fp16 HBM row per node ([s0 64 | s1 64] = 256B).  The two cores of
a pair exchange per-layer node features through a pair-SHARED HBM buffer
(TRN2 LNC1: cores 2k/2k+1 share an HBM domain), synchronized by a 1-element
AllGather barrier per layer.  Per layer, per core:

  m = dinv * (h @ W^T)          PE GEMM (fp16) + DVE mult, node-major SBUF
  scatter-write m -> shared HBM (dma_scatter_add, unique rows, zeroed bufs)
  barrier                       AllGather over pair replica groups
  s = sum_k m[src_k]            transposed dma_gather rounds (feat-major)
                                + DVE adds; self-loops are explicit edges
  h = relu(dinv*s + b)          DVE mult + ACT (feat-major, no transposes)
  g += h^T @ w_l                PE matvecs (fc folded per layer)

head: z_partial = sum_{n in half} g[n] * W1T[n]; host: combine pair halves,
+b1, relu, lin2, log_softmax.
"""
import os
import sys

os.environ.setdefault("NEURON_RT_RESET_CORES", "1")
for _p in ("/opt/trn_rl_repo", "/root/.axon_site/_ro/trn_rl_repo"):
    if os.path.isdir(_p) and _p not in sys.path:
        sys.path.insert(0, _p)

import numpy as np

import concourse.bacc as bacc
import concourse.mybir as mybir
import concourse.tile as tile
from concourse.bass_utils import run_bass_kernel_spmd

P = 128
HID = 64
FIN = 36
NFC = 256
N_CORES = 8
CHUNK = 3968            # gather chunk (SWDGE ring bound), 31*128
SC_CHUNK = 3840         # scatter chunk, 30*128
RG = [[0, 1], [2, 3], [4, 5], [6, 7]]


def preprocess(n, edge_index):
    """Half assignment, rounds, gather streams, scatter indices."""
    src = np.asarray(edge_index[0], dtype=np.int64)
    dst = np.asarray(edge_index[1], dtype=np.int64)

    # augmented graph: self-loop edge per node (PyG GCNConv adds self-loops)
    src_a = np.concatenate([src, np.arange(n)])
    dst_a = np.concatenate([dst, np.arange(n)])
    deg = np.bincount(dst_a, minlength=n)        # == orig in-deg + 1
    dinv = 1.0 / np.sqrt(deg.astype(np.float64))

    # halves: round-robin over in-degree rank; within-half order = rank order
    order = np.argsort(-deg, kind="stable")
    half_of = np.empty(n, dtype=np.int64)
    rank_of = np.empty(n, dtype=np.int64)
    half_of[order] = np.arange(n) % 2
    rank_of[order] = np.arange(n) // 2
    nhalf = [int((half_of == h).sum()) for h in (0, 1)]
    nslot_h = (max(nhalf) + P - 1) // P
    nhp = nslot_h * P                             # padded half size
    ntok = 2 * nhp + P                            # + zero row + token row
    zrow = 2 * nhp                                # all-zero row (dummy gathers)
    trow = ntok - 1                               # barrier token row
    row_of = half_of * nhp + rank_of              # shared-buffer row per node

    # per-half round structure over augmented edges into that half
    streams, seg_lists = [], []
    nks = []
    for h in (0, 1):
        sel = half_of[dst_a] == h
        s_h, d_h = src_a[sel], dst_a[sel]
        dr = rank_of[d_h]
        o = np.argsort(dr, kind="stable")
        s_h, dr = s_h[o], dr[o]
        degs_h = np.bincount(dr, minlength=nhalf[h])
        starts = np.zeros(nhalf[h] + 1, dtype=np.int64)
        np.cumsum(degs_h, out=starts[1:])
        kpos = np.arange(s_h.shape[0]) - starts[dr]
        kmax = int(degs_h.max())
        nk = np.array([int((degs_h > k).sum()) for k in range(kmax)])
        nks.append(nk)
        streams.append((s_h, dr, kpos, nk))
    kmax = max(len(nks[0]), len(nks[1]))
    nk_c = np.zeros(kmax, dtype=np.int64)
    for h in (0, 1):
        nk_c[: len(nks[h])] = np.maximum(nk_c[: len(nks[h])], nks[h])

    L = int(nk_c.sum())
    Lpad = ((L + CHUNK - 1) // CHUNK) * CHUNK
    rstart = np.zeros(kmax + 1, dtype=np.int64)
    np.cumsum(nk_c, out=rstart[1:])

    idx_streams = []
    for h in (0, 1):
        s_h, dr, kpos, nk = streams[h]
        stream = np.full(Lpad, zrow, dtype=np.int64)
        stream[rstart[kpos] + dr] = row_of[s_h]
        idx_streams.append(stream)

    # per-chunk add segments: (gbuf col, s_fm col, len)
    seg_adds = []
    for ci in range(Lpad // CHUNK):
        c0, c1 = ci * CHUNK, min((ci + 1) * CHUNK, L)
        adds = []
        for k in range(kmax):
            lo, hi = max(c0, int(rstart[k])), min(c1, int(rstart[k + 1]))
            if lo < hi:
                adds.append((lo - c0, lo - int(rstart[k]), hi - lo))
        seg_adds.append(adds)

    def wrap16(a):
        cols = a.shape[0] // 16
        w = a.reshape(cols, 16).T.astype(np.int16)
        return np.ascontiguousarray(np.tile(w, (8, 1)))

    idx_w = [wrap16(s) for s in idx_streams]

    sidx_w = []
    for h in (0, 1):
        rows = h * nhp + np.arange(nhp)
        sidx_w.append(wrap16(rows))

    return dict(dinv=dinv, half_of=half_of, rank_of=rank_of, row_of=row_of,
                nhalf=nhalf, nslot_h=nslot_h, nhp=nhp, ntok=ntok, zrow=zrow,
                trow=trow, Lpad=Lpad, seg_adds=seg_adds, idx_w=idx_w,
                sidx_w=sidx_w, nk_c=nk_c, rstart=rstart, L=L)


def build_constants(prep, inputs):
    nhp, nslot_h = prep["nhp"], prep["nslot_h"]
    half_of, rank_of, dinv = prep["half_of"], prep["rank_of"], prep["dinv"]
    n = half_of.shape[0]

    pel_W = np.asarray(inputs["pel_W"], np.float32)
    pel_b = np.asarray(inputs["pel_b"], np.float32)
    pe = pel_W.T + pel_b                      # [N, 32]
    x = np.asarray(inputs["x"], np.float32)   # [8, N, 4]

    node_ids = [np.nonzero(half_of == h)[0] for h in (0, 1)]
    node_ids = [ids[np.argsort(rank_of[ids])] for ids in node_ids]

    Wc = []
    for i in (1, 2, 3):
        w = np.asarray(inputs[f"conv{i}_W"], np.float32).T.astype(np.float16)
        wd = np.zeros((P, HID), np.float16)
        wd[:w.shape[0]] = w
        wd[HID:HID + w.shape[0]] = w
        Wc.append(np.ascontiguousarray(wd))
    bc = [np.asarray(inputs[f"conv{i}_b"], np.float32) for i in (1, 2, 3)]
    bc2 = [np.ascontiguousarray(
        np.concatenate([b, b]).reshape(2 * HID, 1)) for b in bc]

    fc_W = np.asarray(inputs["fc_W"], np.float32).reshape(-1)
    wl = []
    for l in range(3):
        w = fc_W[l::3].astype(np.float16)
        wd = np.zeros((P, 1), np.float16)
        wd[:HID, 0] = w
        wd[HID:, 0] = w
        wl.append(np.ascontiguousarray(wd))
    fc_b = float(np.asarray(inputs["fc_b"], np.float32).reshape(()))

    lin1_W = np.asarray(inputs["lin1_W"], np.float32)
    b1_eff = (np.asarray(inputs["lin1_b"], np.float32)
              + fc_b * lin1_W.sum(axis=1))

    per_core = []
    for c in range(N_CORES):
        h = c % 2
        sA = 2 * (c // 2)
        ids = node_ids[h]
        nh = ids.shape[0]
        x_fm = np.zeros((P, nhp), np.float16)
        for si in range(2):
            xc = np.concatenate([x[sA + si][ids], pe[ids]], axis=1)  # [nh, 36]
            x_fm[si * HID:si * HID + FIN, :nh] = xc.T.astype(np.float16)

        dv = np.zeros(nhp, np.float64)
        dv[:nh] = dinv[ids]
        dinv64 = np.zeros((P, nslot_h, HID), np.float32)
        j = np.arange(nhp)
        dinv64[j % P, j // P, :] = dv[:, None].astype(np.float32)
        dinvb = np.ascontiguousarray(
            np.broadcast_to(dv.astype(np.float16)[None, :], (P, nhp)))

        W1T = np.zeros((nhp, NFC), np.float32)
        W1T[:nh] = lin1_W[:, ids].T
        w1t = np.ascontiguousarray(
            W1T.reshape(nslot_h, P, NFC).transpose(1, 0, 2))  # [P, nslot, NFC]

        per_core.append(dict(x_fm=x_fm, dinv64=np.ascontiguousarray(
            dinv64.reshape(P, nslot_h * HID)), dinvb=dinvb,
            w1t=np.ascontiguousarray(w1t.reshape(P, nslot_h * NFC)),
            gidx=prep["idx_w"][h], sidx=prep["sidx_w"][h]))

    return dict(Wc=Wc, bc2=bc2, wl=wl, b1_eff=b1_eff, per_core=per_core)


def build_program(prep, gbufs=3, ggrp=4, w1grp=4, n_layers=3, n_gchunks=None, use_bar=True, use_scatter=True, use_head=True, use_zero=True, use_gemm=True, use_tail=True, use_gmv=True, use_gmv_skip_mult=False):
    nhp, nslot_h, ntok = prep["nhp"], prep["nslot_h"], prep["ntok"]
    trow, Lpad = prep["trow"], prep["Lpad"]
    seg_adds = prep["seg_adds"]
    f32, f16, i16 = mybir.dt.float32, mybir.dt.float16, mybir.dt.int16
    n_chunks = Lpad // CHUNK
    # tail segmentation: after the chunk ending round k, all s_fm columns
    # >= nk_c[k+1] have received their last add (node col j participates in
    # rounds 0..deg_j-1 and nk_c is the per-round participant count)
    nk_c, rstart, L = prep["nk_c"], prep["rstart"], prep["L"]
    kmax = len(nk_c)
    cut_map = {}
    for frac in (0.45, 0.6, 0.75, 0.87, 0.95):
        tgt = frac * L
        k = int(np.searchsorted(rstart[1:], tgt))
        k = min(k, kmax - 1)
        ci = int((rstart[k + 1] - 1) // CHUNK)
        col = int(nk_c[k + 1]) if k + 1 < kmax else 0
        colb = ((col + P - 1) // P) * P          # block-aligned watermark
        if ci < n_chunks - 1:
            prev = cut_map.get(ci, nhp + 1)
            cut_map[ci] = min(prev, colb)
    n_sc = nhp // SC_CHUNK if nhp % SC_CHUNK == 0 else nhp // SC_CHUNK + 1

    nc = bacc.Bacc("TRN2", debug=False)

    x_dram = nc.dram_tensor("x_fm", [P, nhp], f16, kind="ExternalInput")
    d64_dram = nc.dram_tensor("dinv64", [P, nslot_h * HID], f32,
                              kind="ExternalInput")
    dvb_dram = nc.dram_tensor("dinvb", [P, nhp], f16, kind="ExternalInput")
    Wc_dram = [nc.dram_tensor(f"Wc{i}", [P, HID], f16,
                              kind="ExternalInput") for i in range(3)]
    bc_dram = [nc.dram_tensor(f"bc{i}", [2 * HID, 1], f32,
                              kind="ExternalInput") for i in range(3)]
    wl_dram = [nc.dram_tensor(f"wl{i}", [P, 1], f16, kind="ExternalInput")
               for i in range(3)]
    gidx_dram = nc.dram_tensor("gidx", [P, Lpad // 16], i16,
                               kind="ExternalInput")
    sidx_dram = nc.dram_tensor("sidx", [P, nhp // 16], i16,
                               kind="ExternalInput")
    w1t_dram = nc.dram_tensor("w1t", [P, nslot_h * NFC], f32,
                              kind="ExternalInput")
    z_dram = nc.dram_tensor("z", [1, 2 * NFC], f32, kind="ExternalOutput")

    m_dram = [nc.dram_tensor(f"m_sh{i}", [ntok, 2 * HID], f16,
                             addr_space="Shared") for i in range(3)]
    bar_in = [nc.dram_tensor(f"bar_in{i}", [1, 3 if i == 0 else 1], f16)
              for i in range(4)]
    bar_out = [nc.dram_tensor(f"bar_out{i}", [2, 3 if i == 0 else 1], f16)
               for i in range(4)]

    nblk_tok = ntok // P      # ntok multiple of 128 by construction

    with tile.TileContext(nc) as tc:
        with (
            tc.tile_pool(name="const", bufs=1) as cpool,
            tc.tile_pool(name="state", bufs=1) as spool,
            tc.tile_pool(name="sfm", bufs=1) as fpool,
            tc.tile_pool(name="gath", bufs=gbufs) as gpool,
            tc.tile_pool(name="w1t", bufs=w1grp) as wpool,
            tc.tile_pool(name="psum_t", bufs=2, space="PSUM") as pt_pool,
            tc.tile_pool(name="psum_g", bufs=2, space="PSUM") as pg_pool,
            tc.tile_pool(name="psum_z", bufs=2, space="PSUM") as pz_pool,
        ):
            # ---- constants / resident inputs
            zt = cpool.tile([P, 31, 2 * HID], f16, tag="zt")
            nc.vector.memset(zt[:], 0.0)
            for b in range(3 if use_zero else 0):
                a0 = 0
                while a0 < nblk_tok:
                    an = min(31, nblk_tok - a0)
                    nc.sync.dma_start(
                        out=m_dram[b][:].rearrange("(a p) e -> p a e",
                                                   p=P)[:, a0:a0 + an, :],
                        in_=zt[:, :an, :])
                    a0 += an

            gidx_sb = cpool.tile([P, Lpad // 16], i16, tag="gidx")
            nc.sync.dma_start(out=gidx_sb[:], in_=gidx_dram[:])
            sidx_sb = cpool.tile([P, nhp // 16], i16, tag="sidx")
            nc.sync.dma_start(out=sidx_sb[:], in_=sidx_dram[:])
            x_sb = cpool.tile([P, nhp], f16, tag="x")
            nc.sync.dma_start(out=x_sb[:], in_=x_dram[:])
            d64_sb = cpool.tile([P, nslot_h, HID], f32, tag="d64")
            nc.sync.dma_start(out=d64_sb[:], in_=d64_dram[:].rearrange(
                "p (g f) -> p g f", f=HID))
            dvb_sb = cpool.tile([P, nhp], f16, tag="dvb")
            nc.sync.dma_start(out=dvb_sb[:], in_=dvb_dram[:])
            Wc_sb, bc_sb, wl_sb = [], [], []
            for i in range(3):
                w = cpool.tile([P, HID], f16, tag=f"Wc{i}")
                nc.sync.dma_start(out=w[:], in_=Wc_dram[i][:])
                Wc_sb.append(w)
                b = cpool.tile([2 * HID, 1], f32, tag=f"bc{i}")
                nc.sync.dma_start(out=b[:], in_=bc_dram[i][:])
                bc_sb.append(b)
                wl = cpool.tile([P, 1], f16, tag=f"wl{i}")
                nc.sync.dma_start(out=wl[:], in_=wl_dram[i][:])
                wl_sb.append(wl)

            m_sb = spool.tile([P, nslot_h, 2 * HID], f16, tag="m")
            g_acc = spool.tile([P, 2 * nslot_h], f32, tag="g_acc")
            nc.vector.memset(g_acc[:], 0.0)

            # ---- initial barrier: all cores' zeroing done (reads each buffer)
            if use_bar:
                for b in range(3):
                    nc.sync.dma_start(out=bar_in[0][:, b:b + 1],
                                      in_=m_dram[b][trow:trow + 1, 0:1])
                nc.gpsimd.collective_compute(
                    "AllGather", mybir.AluOpType.bypass, RG,
                    ins=[bar_in[0][:, :]], outs=[bar_out[0][:, :]])
                for b in range(3):
                    nc.sync.dma_start(out=m_dram[b][trow:trow + 1, 1:2],
                                      in_=bar_out[0][0:1, 0:1])

            h_fm = None
            for l in range(n_layers):
                mb = m_dram[l]
                # ---- GEMM + dinv mult -> m_sb (node-major fp16)
                # sample-1 rows copied to a base-0 tile: consecutive matmuls
                # must not alternate base partition (HW fault observed)
                src = x_sb if l == 0 else h_fm
                kk = FIN if l == 0 else HID
                s1 = fpool.tile([HID, nhp], f16, tag="s1")
                nc.vector.tensor_copy(s1[:kk, :], src[HID:HID + kk, :])
                for g0 in range(0, nslot_h if use_gemm else 0, ggrp):
                    gn = min(ggrp, nslot_h - g0)
                    psum_t = pt_pool.tile([P, 2 * ggrp, HID], f32, tag="pt")
                    for si in range(2):
                        for j in range(gn):
                            base = src if si == 0 else s1
                            lhsT = base[si * HID * 0:kk] if False else (
                                src[0:kk, (g0 + j) * P:(g0 + j + 1) * P]
                                if si == 0 else
                                s1[0:kk, (g0 + j) * P:(g0 + j + 1) * P])
                            nc.tensor.matmul(
                                psum_t[:, si * ggrp + j, :], lhsT,
                                Wc_sb[l][0:kk, :],
                                start=True, stop=True)
                    for si in range(2):
                        nc.vector.tensor_mul(
                            m_sb[:, g0:g0 + gn, si * HID:(si + 1) * HID],
                            psum_t[:, si * ggrp:si * ggrp + gn, :],
                            d64_sb[:, g0:g0 + gn, :])
                # ---- publish m to shared HBM (unique rows)
                pos = 0
                while use_scatter and pos < nhp:
                    ln = min(SC_CHUNK, nhp - pos)
                    nc.gpsimd.dma_scatter_add(
                        mb[:, :], m_sb[:, pos // P:(pos + ln) // P, :],
                        sidx_sb[:, pos // 16:(pos + ln) // 16],
                        ln, ln, 2 * HID, single_packet=False)
                    pos += ln
                # ---- barrier: both halves of m published
                if use_bar:
                    nc.sync.dma_start(out=bar_in[l + 1][:, :],
                                      in_=mb[trow:trow + 1, 0:1])
                    nc.gpsimd.collective_compute(
                        "AllGather", mybir.AluOpType.bypass, RG,
                        ins=[bar_in[l + 1][:, :]], outs=[bar_out[l + 1][:, :]])
                    nc.sync.dma_start(out=mb[trow:trow + 1, 1:2],
                                      in_=bar_out[l + 1][0:1, :])
                # ---- gather rounds, accumulate s (feat-major)
                s_fm = fpool.tile([P, nhp], f16, tag="s_fm")
                nc.vector.memset(s_fm[:], 0.0)
                for ci in range(n_chunks if n_gchunks is None else min(n_gchunks, n_chunks)):
                    gbuf = gpool.tile([P, 1, CHUNK], f16, tag="gb")
                    nc.gpsimd.dma_gather(
                        gbuf[:, :, :], mb[:, :],
                        gidx_sb[:, ci * (CHUNK // 16):(ci + 1) * (CHUNK // 16)],
                        CHUNK, CHUNK, 2 * HID, transpose=True,
                        single_packet=False)
                    for (gc, sc, ln) in seg_adds[ci]:
                        nc.vector.tensor_add(s_fm[:, sc:sc + ln],
                                             s_fm[:, sc:sc + ln],
                                             gbuf[:, 0, gc:gc + ln])
                # ---- tail: dinv mult + relu(bias)
                h_fm = fpool.tile([P, nhp], f16, tag="h_fm")
                if use_tail:
                    nc.vector.tensor_mul(s_fm[:], s_fm[:], dvb_sb[:])
                    nc.scalar.activation(h_fm[:], s_fm[:],
                                         mybir.ActivationFunctionType.Relu,
                                         bias=bc_sb[l][:], scale=1.0)
                else:
                    nc.vector.memset(h_fm[:], 0.0)
                # ---- fc fold: g += h^T w_l
                psum_g = pg_pool.tile([P, 2 * nslot_h], f32, tag="pg")
                h1 = fpool.tile([HID, nhp], f16, tag="h1")
                nc.vector.tensor_copy(h1[:], h_fm[HID:2 * HID, :])
                for si in range(2 if use_gmv else 0):
                    for j in range(nslot_h):
                        hl = (h_fm[0:HID, j * P:(j + 1) * P] if si == 0
                              else h1[:, j * P:(j + 1) * P])
                        nc.tensor.matmul(
                            psum_g[:, si * nslot_h + j:si * nslot_h + j + 1],
                            hl, wl_sb[l][0:HID, :],
                            start=True, stop=True)
                if use_gmv:
                    nc.vector.tensor_add(g_acc[:], g_acc[:], psum_g[:])

            # ---- head: z_partial[s] = sum_blocks g^T W1T
            z_acc = spool.tile([1, 2 * NFC], f32, tag="z_acc")
            nc.vector.memset(z_acc[:], 0.0)
            for g0 in range(0, nslot_h if use_head else 0, w1grp):
                gn = min(w1grp, nslot_h - g0)
                w1t = wpool.tile([P, w1grp, NFC], f32, tag="w1t")
                nc.sync.dma_start(
                    out=w1t[:, :gn, :],
                    in_=w1t_dram[:].rearrange("p (g f) -> p g f",
                                              f=NFC)[:, g0:g0 + gn, :])
                for si in range(2):
                    psum_z = pz_pool.tile([1, NFC], f32, tag="pz")
                    for j in range(gn):
                        jj = si * nslot_h + g0 + j
                        nc.tensor.matmul(
                            psum_z[:], g_acc[:, jj:jj + 1], w1t[:, j, :],
                            start=(j == 0), stop=(j == gn - 1))
                    nc.vector.tensor_add(z_acc[:, si * NFC:(si + 1) * NFC],
                                         z_acc[:, si * NFC:(si + 1) * NFC],
                                         psum_z[:])
            nc.sync.dma_start(out=z_dram[:], in_=z_acc[:])

    nc.compile()
    return nc


def make_in_maps(prep, consts):
    shared = {}
    for i in range(3):
        shared[f"Wc{i}"] = consts["Wc"][i]
        shared[f"bc{i}"] = consts["bc2"][i]
        shared[f"wl{i}"] = consts["wl"][i]
    maps = []
    for c in range(N_CORES):
        pc = consts["per_core"][c]
        maps.append(dict(shared, x_fm=pc["x_fm"], dinv64=pc["dinv64"],
                         dinvb=pc["dinvb"], w1t=pc["w1t"],
                         gidx=pc["gidx"], sidx=pc["sidx"]))
    return maps


def finish_host(z_pairs, consts, inputs):
    """z_pairs: [8, 2, 256] per-core partials (core order)."""
    b1 = consts["b1_eff"]
    W2 = np.asarray(inputs["lin2_W"], np.float32)
    b2 = np.asarray(inputs["lin2_b"], np.float32)
    z = np.zeros((8, NFC), np.float32)
    for p in range(4):
        for si in range(2):
            z[2 * p + si] = z_pairs[2 * p][si] + z_pairs[2 * p + 1][si] + b1
    z = np.maximum(z, 0.0)
    logits = z @ W2.T + b2
    mx = logits.max(axis=1, keepdims=True)
    e = np.exp(logits - mx)
    return ((logits - mx) - np.log(e.sum(axis=1, keepdims=True))).astype(
        np.float32)


_PROGRAM_CACHE = {}


def _get_program(prep, cache_key):
    hit = _PROGRAM_CACHE.get(cache_key)
    if hit is None:
        hit = build_program(prep)
        _PROGRAM_CACHE[cache_key] = hit
    return hit


def kernel(**inputs) -> np.ndarray:
    x = np.asarray(inputs["x"])
    bs, n = x.shape[0], x.shape[1]
    assert bs == N_CORES, f"expected batch {N_CORES}, got {bs}"

    edge_index = np.asarray(inputs["edge_index"])
    prep = preprocess(n, edge_index)
    cache_key = (n, edge_index.shape[1], hash(edge_index.tobytes()))
    nc = _get_program(prep, cache_key)
    consts = build_constants(prep, inputs)
    in_maps = make_in_maps(prep, consts)

    last_err = None
    for attempt in range(3):
        try:
            res = run_bass_kernel_spmd(nc, in_maps, list(range(N_CORES)))
            break
        except Exception as e:
            last_err = e
    else:
        raise last_err

    z_pairs = np.stack([res.results[c]["z"].reshape(2, NFC)
                        for c in range(N_CORES)])
    return finish_host(z_pairs, consts, inputs)
